# revision 57
# baseline (speedup 1.0000x reference)
"""Trainium2 Bass kernel for nn_CGLayer (gnn_message_passing).

Contract: kernel(**inputs) takes FULL inputs (as reference.setup_inputs()),
returns FULL output [8,128,1,16,9] f32. Internally: data-parallel over the
batch dim across 8 NeuronCores; per core one batch element.

Algebraic reduction (exact):
  X   = conn @ vertices                  (message passing, per batch)
  Y   = mix_nl(cg(X, X))                 (per-node quadratic in X)
  S   = sum_j sph[:, j, :]               (neighbor sum commutes through the
  Z   = mix_rel(cg(Y, S))                 relative-CG stage: x-side is
  out = Z / sqrt(sum Z^2 / 16)            j-independent)

Device pipeline per core — everything node(i)-on-partition:
  A:  X[i,144]     = matmul(lhsT=connT, rhs=vcat), fp32
  S:  Ssum[i,9]    = reduce_j(sph)                       (gpsimd)
  B:  P[i,9984]    = 13 stride-0 DVE pair products, bf16 out, packed
                     symmetry-folded slot layout (W2 host-folds CG x w_nl)
      PT chunks    = DMA xbar transpose (no PE involvement)
      Y^T[i,144]   = 78 bf16 matmuls lhsT=PT-chunk rhs=W2-chunk, PSUM-accum
  C:  P2[i,1296]   = Y^T * Ssum broadcast (1 DVE op), bf16, padded to 1408
      P2T chunks   = DMA xbar transpose
      Z^T[i,144]   = 11 bf16 matmuls lhsT=P2T-chunk rhs=W3-chunk
Host epilogue: unpack e=(l,c,k) columns, global per-l normalization.
"""
import numpy as np
import ml_dtypes
from math import factorial, sqrt

MAXL = 2
CH = 16
NN = 128
NB = 8
LDIM = [1, 3, 5]
FOFF = [0, 16, 64]
NF = 144
SOFF = [0, 1, 4]

# ------------------------------------------------------------- CG tables
def _cg_coeff(j1, m1, j2, m2, j3, m3):
    if m3 != m1 + m2:
        return 0.0
    pre = sqrt((2 * j3 + 1) * factorial(j3 + j1 - j2) * factorial(j3 - j1 + j2)
               * factorial(j1 + j2 - j3) / factorial(j1 + j2 + j3 + 1))
    pre *= sqrt(factorial(j3 + m3) * factorial(j3 - m3) * factorial(j1 - m1)
                * factorial(j1 + m1) * factorial(j2 - m2) * factorial(j2 + m2))
    s = 0.0
    vmin = max(0, j2 - j3 - m1, j1 - j3 + m2)
    vmax = min(j1 + j2 - j3, j1 - m1, j2 + m2)
    for v in range(vmin, vmax + 1):
        s += (-1) ** v / (factorial(v) * factorial(j1 + j2 - j3 - v)
                          * factorial(j1 - m1 - v) * factorial(j2 + m2 - v)
                          * factorial(j3 - j2 + m1 + v) * factorial(j3 - j1 - m2 + v))
    return pre * s


def _cg_matrix(l1, l2, l):
    M = np.zeros((2 * l1 + 1, 2 * l2 + 1, 2 * l + 1))
    for m1 in range(-l1, l1 + 1):
        for m2 in range(-l2, l2 + 1):
            if -l <= m1 + m2 <= l:
                M[m1 + l1, m2 + l2, m1 + m2 + l] = _cg_coeff(l1, m1, l2, m2, l, m1 + m2)
    return M


def _valid_pairs(l):
    return [(l1, l2) for l1 in range(3) for l2 in range(3)
            if abs(l1 - l2) <= l <= l1 + l2]

# ----------------------------------------------------- packed slot layout
# q = (l1, l2, m1) with l1 <= l2; for diagonal pairs m2 >= m1 (symmetric
# fold: the (m2, m1) ordering's weight folds onto the kept slot with the
# channel grid transposed). Slots of one q are contiguous over its valid,
# contiguous m2-range; each (q, m2) block is a 256-slot (c, d) grid.
def _build_qfold():
    q = []
    off = 0
    for l1 in range(3):
        for l2 in range(l1, 3):
            for m1 in range(2 * l1 + 1):
                mt1 = m1 - l1
                lo = max(0, -2 - mt1 + l2)
                hi = min(2 * l2, 2 - mt1 + l2)
                if l1 == l2:
                    lo = max(lo, m1)
                if lo > hi:
                    continue
                n = hi - lo + 1
                q.append(dict(l1=l1, l2=l2, m1=m1, m2_lo=lo, n_m2=n, off=off))
                off += 256 * n
    return q, off

Q_FOLD, NSLOT = _build_qfold()          # 13 ops, 9984 slots
NCHUNK = NSLOT // 128                   # 78
_QIDX = {(e["l1"], e["l2"], e["m1"]): e for e in Q_FOLD}
# pipeline groups in chunks (256-slot aligned; product ops split at bounds)
GCHUNKS = [8, 10, 12, 8, 10, 14, 16]
GBOUND = [0]
for c in GCHUNKS:
    GBOUND.append(GBOUND[-1] + 128 * c)
assert GBOUND[-1] == NSLOT


def _group_ops():
    gops = [[] for _ in GCHUNKS]
    for gi in range(len(GCHUNKS)):
        a, b = GBOUND[gi], GBOUND[gi + 1]
        for e in Q_FOLD:
            s0, s1 = e["off"], e["off"] + 256 * e["n_m2"]
            lo, hi = max(a, s0), min(b, s1)
            if lo >= hi:
                continue
            j0 = (lo - s0) // 256
            j1 = (hi - s0) // 256
            gops[gi].append(dict(l1=e["l1"], l2=e["l2"], m1=e["m1"],
                                 m2_lo=e["m2_lo"] + j0, n_m2=j1 - j0, off=lo))
    return gops

G_OPS = _group_ops()

# Y column layout is s-group-major: col = YOFF[g] + (l - |g-2|)*16 + c'.
# Each 256-slot (q, m2) block then feeds ONE contiguous <=48-col range, so
# W2 is stored slim ([NSLOT, 48] padded) instead of [NSLOT, 144] dense.
SG_NCOL = [16, 32, 48, 32, 16]
YOFF = [0, 16, 48, 96, 128]
NW2 = 48


def _ycol(l, m):
    g = (m - l) + 2
    return YOFF[g] + (l - abs(g - 2)) * 16


def _chunk_meta():
    meta = []
    for e in Q_FOLD:
        mt1 = e["m1"] - e["l1"]
        for j in range(e["n_m2"]):
            g = mt1 + (e["m2_lo"] + j - e["l2"]) + 2
            ncol = 16 * (3 - abs(g - 2))
            for _ in range(2):                      # 2 chunks per 256-block
                meta.append((YOFF[g], ncol, g))
    # start/stop per g-group (first/last chunk writing that column range)
    first, last = {}, {}
    for k, (_, _, g) in enumerate(meta):
        first.setdefault(g, k)
        last[g] = k
    return [(gc0, ncol, first[g] == k, last[g] == k)
            for k, (gc0, ncol, g) in enumerate(meta)]

CHUNK_META = _chunk_meta()

NP2 = 9 * NF                            # 1296
NP2PAD = 1408                           # 11 chunks of 128
NCH3 = NP2PAD // 128

_CAR, _DAR = np.meshgrid(np.arange(16), np.arange(16), indexing="ij")


def _assemble_W2(w_nl):
    """W2[NSLOT, 144] f64: folded CG x w_nl; cols e = FOFF[l]+c'*LDIM[l]+k."""
    W2 = np.zeros((NSLOT, NF))
    for l in range(3):
        off_t = 0
        for (p1, p2) in _valid_pairs(l):
            Cg = _cg_matrix(p1, p2, l)
            wl = np.asarray(w_nl[l], np.float64)
            for m1 in range(2 * p1 + 1):
                for m2 in range(2 * p2 + 1):
                    st = (m1 - p1) + (m2 - p2)
                    if abs(st) > l:
                        continue
                    gc = Cg[m1, m2, st + l]
                    if gc == 0.0:
                        continue
                    if (p1 < p2) or (p1 == p2 and m1 <= m2):
                        e_ = _QIDX[(p1, p2, m1)]
                        base = e_["off"] + (m2 - e_["m2_lo"]) * 256
                        slots = base + _CAR * 16 + _DAR
                    else:
                        e_ = _QIDX[(p2, p1, m2)]
                        base = e_["off"] + (m1 - e_["m2_lo"]) * 256
                        slots = base + _DAR * 16 + _CAR
                    t = off_t + _CAR * 16 + _DAR
                    cols = YOFF[st + 2] + (l - abs(st)) * 16 + np.arange(16)
                    W2[np.ix_(slots.ravel(), cols)] += gc * wl[t.ravel(), :]
            off_t += 256
    return W2


def _assemble_W3(w_rel):
    """W3[NP2PAD, 144]: contraction P2[i,(n,e)] -> Z[i,e']; rows n*144+e."""
    W3 = np.zeros((NP2PAD, NF))
    ar = np.arange(16)
    for l in range(3):
        off_t = 0
        for (p1, p2) in _valid_pairs(l):          # p1 = Y side, p2 = sph side
            Cg = _cg_matrix(p1, p2, l)
            wr = np.asarray(w_rel[l], np.float64)
            for m1 in range(2 * p1 + 1):
                for m2 in range(2 * p2 + 1):
                    st = (m1 - p1) + (m2 - p2)
                    if abs(st) > l:
                        continue
                    gc = Cg[m1, m2, st + l]
                    if gc == 0.0:
                        continue
                    n = SOFF[p2] + m2
                    rows = n * NF + _ycol(p1, m1) + ar
                    cols = FOFF[l] + ar * LDIM[l] + (st + l)
                    W3[np.ix_(rows, cols)] += gc * wr[off_t:off_t + 16, :]
            off_t += 16
    return W3

# ------------------------------------------------------------ bass builder
_NC_CACHE = {}


def _build_nc(debug=False):
    import concourse.bacc as bacc
    import concourse.bass as bass
    import concourse.tile as tile
    from concourse import mybir

    f32 = mybir.dt.float32
    bf16 = mybir.dt.bfloat16
    nc = bacc.Bacc()
    d_cv = nc.declare_dram_parameter("cv", [128, 128 + NF], f32, isOutput=False)
    d_sph = nc.declare_dram_parameter("sph", [128, 128 * 9], bf16, isOutput=False)
    d_w2 = nc.declare_dram_parameter("w2", [128, NCHUNK * NW2], bf16, isOutput=False)
    d_w3 = nc.declare_dram_parameter("w3", [128, NCH3 * NF], bf16, isOutput=False)
    d_zout = nc.declare_dram_parameter("zout", [128, NF], f32, isOutput=True)
    if debug:
        d_dbgx = nc.declare_dram_parameter("dbgx", [128, NF], bf16, isOutput=True)
        d_dbgs = nc.declare_dram_parameter("dbgs", [128, 9], f32, isOutput=True)
        d_dbgp = nc.declare_dram_parameter("dbgp", [128, NSLOT], bf16, isOutput=True)
        d_dbgy = nc.declare_dram_parameter("dbgy", [128, NF], f32, isOutput=True)
        d_dbgp2 = nc.declare_dram_parameter("dbgp2", [128, NP2PAD], bf16, isOutput=True)

    def vap(t, doff, freedims):
        base = t[:] if not isinstance(t, bass.AP) else t
        return bass.AP(tensor=base.tensor, offset=base.offset + doff,
                       ap=[list(base.ap[0])] + [list(d) for d in freedims])

    from concourse.masks import make_identity

    with tile.TileContext(nc) as tc:
      with (
        tc.tile_pool(name="sb", bufs=1) as sb,
        tc.tile_pool(name="pp", bufs=7) as pp,
        tc.tile_pool(name="ptp", bufs=7) as ptp,
        tc.tile_pool(name="ps_a", bufs=1, space="PSUM") as ps_a,
        tc.tile_pool(name="ps_y", bufs=1, space="PSUM") as ps_y,
      ):
        # ---- input DMAs split across the two HWDGE dispatchers (sync/scalar)
        cv = sb.tile([128, 128 + NF], f32)              # connT | vcat fused
        nc.sync.dma_start(out=cv, in_=d_cv[:, :])
        sph = sb.tile([128, 128 * 9], bf16)
        nc.sync.dma_start(out=sph, in_=d_sph[:, :])
        # regular DMAs on scalar; ALL xbar transposes on sync (the transpose
        # crossbar is a shared unit — concurrent use from both dispatchers
        # produced flaky corruption)
        w2 = sb.tile([128, NCHUNK, NW2], bf16)
        wsplit = [0, 13, 26, 39, 52, 65, NCHUNK]
        for g in range(6):
            s, e = wsplit[g] * NW2, wsplit[g + 1] * NW2
            nc.scalar.dma_start(out=vap(w2, s, [[1, e - s]]), in_=d_w2[:, s:e])
        w3 = sb.tile([128, NCH3, NF], bf16)
        nc.sync.dma_start(
            out=w3, in_=d_w3[:, :].rearrange("p (c e) -> p c e", c=NCH3, e=NF))


        # ---- stage A: X[i, feat] = connT.T @ vcat, cast to bf16
        x_ps = ps_a.tile([128, NF], f32)
        nc.tensor.matmul(x_ps, cv[:, 0:128], cv[:, 128:128 + NF],
                         start=True, stop=True)
        X = sb.tile([128, NF], bf16)
        nc.scalar.activation(X, x_ps, mybir.ActivationFunctionType.Copy)

        # ---- stage B: per-q products (DVE/gpsimd, bf16 2x) -> per-q DMA
        # transpose (alternating dispatch engines) -> 78 pipelined matmuls.
        # One PSUM bank per s-group: matmul start=True clears the whole
        # bank, so independent accumulation ranges must not share one.
        ymixg = [ps_y.tile([128, SG_NCOL[g]], f32, name=f"ymix{g}")
                 for g in range(5)]
        for gi, gops in enumerate(G_OPS):
            gbase = GBOUND[gi]
            gend = GBOUND[gi + 1]
            gslots = gend - gbase
            nch = gslots // 128
            P = pp.tile([128, 2048], bf16)
            for op in gops:
                l1, l2, m1 = op["l1"], op["l2"], op["m1"]
                nm2 = op["n_m2"]
                peng = nc.vector
                peng.tensor_tensor(
                    out=vap(P, op["off"] - gbase,
                            [[256, nm2], [16, 16], [1, 16]]),
                    in0=vap(X, FOFF[l1] + m1,
                            [[0, nm2], [LDIM[l1], 16], [0, 16]]),
                    in1=vap(X, FOFF[l2] + op["m2_lo"],
                            [[1, nm2], [0, 16], [LDIM[l2], 16]]),
                    op=mybir.AluOpType.mult)
            PT = ptp.tile([128, 16, 128], bf16)
            nc.sync.dma_start(out=PT[:, 0:nch, :], in_=P[:, 0:gslots],
                              transpose=True)
            if debug:
                nc.sync.dma_start(out=d_dbgp[:, gbase:gend], in_=P[:, 0:gslots])
            for c in range(nch):
                k = gbase // 128 + c
                gc0, ncol, st_f, sp_f = CHUNK_META[k]
                gidx = YOFF.index(gc0)
                nc.tensor.matmul(ymixg[gidx], PT[:, c, :],
                                 w2[:, k, 0:ncol], start=st_f, stop=sp_f)

        # ---- stage S: Ssum[i, 9] = sum_j sph (bf16; emitted after products)
        ssum = sb.tile([128, 9], f32)
        nc.vector.tensor_reduce(
            ssum, vap(sph, 0, [[1, 9], [9, 128]]),
            mybir.AxisListType.X, mybir.AluOpType.add)

        # ---- stage C: P2 = Y^T * Ssum -> PE transposes -> 11 matmuls
        # barrier: P2 reads the ymix PSUM accumulators; a mid-accumulation
        # read corrupts (observed as flaky warm-run failures)
        tc.strict_bb_all_engine_barrier()
        P2 = sb.tile([128, NP2PAD], bf16)
        nc.gpsimd.memset(P2[:, NP2:NP2PAD], 0.0)
        for g in range(5):
            ncol = SG_NCOL[g]
            nc.vector.tensor_tensor(
                out=vap(P2, YOFF[g], [[NF, 9], [1, ncol]]),
                in0=vap(ymixg[g], 0, [[0, 9], [1, ncol]]),
                in1=vap(ssum, 0, [[1, 9], [0, ncol]]),
                op=mybir.AluOpType.mult)
        pt3 = sb.tile([128, NCH3, 128], bf16)
        nc.sync.dma_start(out=pt3, in_=P2[:, :], transpose=True)
        z_ps = x_ps                     # bank reuse: X consumed long ago
        for c in range(NCH3):
            nc.tensor.matmul(z_ps, pt3[:, c, :], w3[:, c, :],
                             start=(c == 0), stop=(c == NCH3 - 1))
        tc.strict_bb_all_engine_barrier()
        zsb = sb.tile([128, NF], f32)
        nc.scalar.activation(zsb, z_ps, mybir.ActivationFunctionType.Copy)
        nc.sync.dma_start(out=d_zout[:, :], in_=zsb)

        if debug:
            nc.sync.dma_start(out=d_dbgx[:, :], in_=X)
            nc.sync.dma_start(out=d_dbgs[:, :], in_=ssum)
            ydbg = sb.tile([128, NF], f32)
            for g in range(5):
                nc.vector.tensor_copy(
                    out=ydbg[:, YOFF[g]:YOFF[g] + SG_NCOL[g]], in_=ymixg[g])
            nc.sync.dma_start(out=d_dbgy[:, :], in_=ydbg)
            nc.sync.dma_start(out=d_dbgp2[:, :], in_=P2)

    nc.compile()
    return nc

# ------------------------------------------------------------- host entry
LAST_RESULT = {}


def _get_nc():
    if "nc" not in _NC_CACHE:
        _NC_CACHE["nc"] = _build_nc()
    return _NC_CACHE["nc"]


def _pack_chunked(W, nchunk):
    """[nchunk*128, e] -> [128, nchunk*e] bf16 (chunk-major per partition)."""
    e = W.shape[1]
    return np.ascontiguousarray(
        W.reshape(nchunk, 128, e).transpose(1, 0, 2)
        .astype(ml_dtypes.bfloat16).reshape(128, nchunk * e))


def kernel(vertices_0, vertices_1, vertices_2, connectivity,
           sph_0, sph_1, sph_2,
           w_nl_0, w_nl_1, w_nl_2,
           w_rel_0, w_rel_1, w_rel_2):
    from concourse.bass_utils import run_bass_kernel_spmd

    f = np.float32
    verts = [np.asarray(v, f) for v in (vertices_0, vertices_1, vertices_2)]
    sphs = [np.asarray(s, f) for s in (sph_0, sph_1, sph_2)]
    conn = np.asarray(connectivity)
    W2 = _assemble_W2([np.asarray(w, f) for w in (w_nl_0, w_nl_1, w_nl_2)])
    W3 = _assemble_W3([np.asarray(w, f) for w in (w_rel_0, w_rel_1, w_rel_2)])
    # slim W2: per chunk keep only its <=48 live columns (zero-padded to 48)
    W2s = np.zeros((NSLOT, NW2))
    for k, (gc0, ncol, _, _) in enumerate(CHUNK_META):
        W2s[k * 128:(k + 1) * 128, 0:ncol] = W2[k * 128:(k + 1) * 128,
                                                gc0:gc0 + ncol]
    w2p = _pack_chunked(W2s, NCHUNK)
    w3p = _pack_chunked(W3, NCH3)

    in_maps = []
    for b in range(NB):
        cv = np.concatenate(
            [conn[b].astype(f).T] + [v[b].reshape(128, -1) for v in verts], axis=1)
        sph_cat = np.concatenate([s[b][:, :, 0, :] for s in sphs], axis=-1)
        sph_bf = sph_cat.reshape(128, 128 * 9).astype(ml_dtypes.bfloat16)
        in_maps.append(dict(cv=np.ascontiguousarray(cv),
                            sph=np.ascontiguousarray(sph_bf),
                            w2=w2p, w3=w3p))

    res = run_bass_kernel_spmd(_get_nc(), in_maps, list(range(NB)))
    LAST_RESULT["res"] = res
    Z = np.stack([res.results[b]["zout"] for b in range(NB)])   # [8, 128, 144]

    # host epilogue: unpack e=(l,c,k) cols, global per-l normalization
    out = np.zeros((NB, 128, 1, 16, 9), dtype=f)
    koff = [0, 1, 4]
    for l in range(3):
        cols = FOFF[l] + (np.arange(16)[:, None] * LDIM[l]
                          + np.arange(LDIM[l])[None, :])
        blk = Z[:, :, cols]                                     # [8,128,16,ld]
        nf = np.sum(blk.astype(np.float64) ** 2)
        out[:, :, 0, :, koff[l]:koff[l] + LDIM[l]] = blk / np.sqrt(nf / 16.0)
    return out
